# revision 20
# baseline (speedup 1.0000x reference)
"""Trainium2 Bass kernel for CudaMorphUnpool2D (max-unpool scatter + 3x3 dilation).

Strategy:
  - 1024 (b,c) planes sharded 128/core across 8 NeuronCores (fully data parallel).
  - Host prep: the unpool scatter (pure data movement, last-writer-wins) is folded
    into input marshaling: the 256x256 canvas is built per plane with one numpy
    fancy-assignment and shipped as 4 parity-quadrant planes (even/odd row x
    even/odd col), fp16, padded to 132 columns with -inf guards so each slab's
    row-block is a single contiguous DMA descriptor per partition.
  - Device: separable 3x3 windowed max entirely with 2x-rate DVE tensor_tensor
    MAX ops (fp16, stride-1, 4B-aligned APs).  The 2-byte-misaligned column
    shifts are materialized by the Scalar (ACT) engine, which is otherwise idle.
    Outputs stay parity-planar (even rows / odd rows; cols planar within) and
    are re-interleaved on the host during the gather step.
  - Out-of-canvas window taps use -65504 (fp16 lowest) guards to exactly match
    the reference's -inf padding semantics at the borders.

Measured on trn2 (8 cores): 132.9us vs 338.3us baseline (2.55x), rel err 3.7e-4.
DVE busy ~111us (the structural floor for this decomposition: 3.09 MAX-op
elements per output element at 2 elem/cycle); DMA ~34MB/core ~94us aggregate.
"""
import os
import sys
import numpy as np
from contextlib import ExitStack

H, W = 256, 256
HP, WP = 128, 128
# quadrant rows per slab (out rows = 2*si); small first slab starts DVE ~2us
# sooner (less data gates the first colmax), small last slab shrinks the final
# out-DMA drain; slab count is kept at 9 (each extra slab costs ~0.9us DVE)
SLABS = [8, 16, 16, 16, 16, 16, 16, 16, 8]
assert sum(SLABS) == HP
MX = max(SLABS) + 1
NCORES = 8
PPC = 128               # planes per core
NEG = -65504.0          # fp16 lowest: stands in for the reference's -inf pad

for _p in ("/opt/trn_rl_repo", "/root/.axon_site/_ro/trn_rl_repo"):
    if os.path.isdir(_p) and _p not in sys.path:
        sys.path.append(_p)


def _build_nc():
    import concourse.bass as bass  # noqa: F401
    import concourse.tile as tile
    from concourse import bacc, mybir

    dt = mybir.dt.float16
    AO = mybir.AluOpType

    nc = bacc.Bacc("TRN2", target_bir_lowering=False, debug=False)
    # quadrant canvases, host-padded to 132 cols (data in [2:130], NEG guards
    # elsewhere) so a whole slab row-block is one contiguous DMA per partition:
    # q<rowparity><colparity>[p, a, 2+b] = canvas[2a+rp, 2b+cp]
    qee = nc.dram_tensor("qee", [PPC, HP, 132], dt, kind="ExternalInput").ap()
    qeo = nc.dram_tensor("qeo", [PPC, HP, 132], dt, kind="ExternalInput").ap()
    qoe = nc.dram_tensor("qoe", [PPC, HP, 132], dt, kind="ExternalInput").ap()
    qoo = nc.dram_tensor("qoo", [PPC, HP, 132], dt, kind="ExternalInput").ap()
    # outputs: planar parity rows; cols 0:128 = even out cols, 128:256 = odd
    oE = nc.dram_tensor("oE", [PPC, HP, 2 * WP], dt, kind="ExternalOutput").ap()
    oO = nc.dram_tensor("oO", [PPC, HP, 2 * WP], dt, kind="ExternalOutput").ap()

    with tile.TileContext(nc) as tc, ExitStack() as ctx:
        pin = ctx.enter_context(tc.tile_pool(name="pin", bufs=2))
        psh = ctx.enter_context(tc.tile_pool(name="psh", bufs=2))
        pp = ctx.enter_context(tc.tile_pool(name="pp", bufs=2))
        pcm = ctx.enter_context(tc.tile_pool(name="pcm", bufs=2))
        ps = ctx.enter_context(tc.tile_pool(name="ps", bufs=2))
        pout = ctx.enter_context(tc.tile_pool(name="pout", bufs=2))

        i0 = 0
        for si in SLABS:
            R = si + 1
            # --- input tiles: data in cols [2:130]; guard cols 0:2 & 130:132.
            # E-plane tile rows t in [0,si]  <->  quadrant row a = i0 + t
            # O-plane tile rows t in [0,si]  <->  quadrant row a = i0 - 1 + t
            QEE = pin.tile([128, MX, 132], dt, tag="QEE")
            QEO = pin.tile([128, MX, 132], dt, tag="QEO")
            QOE = pin.tile([128, MX, 132], dt, tag="QOE")
            QOO = pin.tile([128, MX, 132], dt, tag="QOO")
            # E rows: a in [i0, i0+si]; last slab: a=HP row is out-of-canvas
            e_hi = min(HP, i0 + si + 1)
            n_e = e_hi - i0
            if n_e < R:
                nc.gpsimd.memset(QEE[:, n_e:R, :], NEG)
                nc.gpsimd.memset(QEO[:, n_e:R, :], NEG)
            nc.sync.dma_start(QEE[:, 0:n_e, :], qee[:, i0:e_hi, :])
            nc.sync.dma_start(QEO[:, 0:n_e, :], qeo[:, i0:e_hi, :])
            # O rows: a in [i0-1, i0+si-1]; first slab: a=-1 is out-of-canvas
            o_lo = max(0, i0 - 1)
            t0 = o_lo - (i0 - 1)
            if t0 > 0:
                nc.gpsimd.memset(QOE[:, 0:t0, :], NEG)
                nc.gpsimd.memset(QOO[:, 0:t0, :], NEG)
            nc.sync.dma_start(QOE[:, t0:R, :], qoe[:, o_lo:i0 + si, :])
            nc.sync.dma_start(QOO[:, t0:R, :], qoo[:, o_lo:i0 + si, :])

            # --- ACT: 4B-realigned column-shifted copies (odd-elem offsets)
            shEO = psh.tile([128, MX, 128], dt, tag="shEO")  # O[b-1], even rows
            shEE = psh.tile([128, MX, 128], dt, tag="shEE")  # E[b+1], even rows
            shOO = psh.tile([128, MX, 128], dt, tag="shOO")  # O[b-1], odd rows
            shOE = psh.tile([128, MX, 128], dt, tag="shOE")  # E[b+1], odd rows
            nc.scalar.copy(shEO[:, 0:R, :], QEO[:, 0:R, 1:129])
            nc.scalar.copy(shEE[:, 0:R, :], QEE[:, 0:R, 3:131])
            nc.scalar.copy(shOO[:, 0:R, :], QOO[:, 0:R, 1:129])
            nc.scalar.copy(shOE[:, 0:R, :], QOE[:, 0:R, 3:131])

            # --- colmax (all DVE MAX at 2x): cm[p, t, 0:128]=even out cols,
            # cm[p, t, 128:256]=odd out cols
            P_e = pp.tile([128, MX, 128], dt, tag="P_e")
            P_o = pp.tile([128, MX, 128], dt, tag="P_o")
            nc.vector.tensor_tensor(P_e[:, 0:R, :], QEE[:, 0:R, 2:130], QEO[:, 0:R, 2:130], AO.max)
            nc.vector.tensor_tensor(P_o[:, 0:R, :], QOE[:, 0:R, 2:130], QOO[:, 0:R, 2:130], AO.max)
            cmE = pcm.tile([128, MX, 256], dt, tag="cmE")
            cmO = pcm.tile([128, MX, 256], dt, tag="cmO")
            nc.vector.tensor_tensor(cmE[:, 0:R, 0:128], shEO[:, 0:R, :], P_e[:, 0:R, :], AO.max)
            nc.vector.tensor_tensor(cmE[:, 0:R, 128:256], P_e[:, 0:R, :], shEE[:, 0:R, :], AO.max)
            nc.vector.tensor_tensor(cmO[:, 0:R, 0:128], shOO[:, 0:R, :], P_o[:, 0:R, :], AO.max)
            nc.vector.tensor_tensor(cmO[:, 0:R, 128:256], P_o[:, 0:R, :], shOE[:, 0:R, :], AO.max)

            # --- rowmax: out even row 2a = max(cmO[a-1], cmE[a], cmO[a])
            #             out odd  row 2a+1 = max(cmE[a], cmO[a], cmE[a+1])
            # tile idx: cmE[u] <-> a=i0+u ; cmO[u] <-> a=i0-1+u
            S = ps.tile([128, MX - 1, 256], dt, tag="S")
            outE = pout.tile([128, MX - 1, 256], dt, tag="outE")
            outO = pout.tile([128, MX - 1, 256], dt, tag="outO")
            nc.vector.tensor_tensor(S[:, 0:si, :], cmE[:, 0:si, :], cmO[:, 1:si + 1, :], AO.max)
            nc.vector.tensor_tensor(outE[:, 0:si, :], cmO[:, 0:si, :], S[:, 0:si, :], AO.max)
            nc.vector.tensor_tensor(outO[:, 0:si, :], S[:, 0:si, :], cmE[:, 1:si + 1, :], AO.max)

            nc.sync.dma_start(oE[:, i0:i0 + si, :], outE[:, 0:si, :])
            nc.sync.dma_start(oO[:, i0:i0 + si, :], outO[:, 0:si, :])
            i0 += si

    nc.compile()
    return nc


_NC_CACHE = {}


def _get_nc():
    if "nc" not in _NC_CACHE:
        _NC_CACHE["nc"] = _build_nc()
    return _NC_CACHE["nc"]


def _prep_in_maps(f, p):
    """Host prep: unpool-scatter into the canvas (last-writer-wins, matching the
    reference's row-major duplicate-index semantics), split into parity
    quadrants, shard across cores."""
    BC = f.shape[0] * f.shape[1]
    fv = f.reshape(BC, HP * WP).astype(np.float16)
    idx = p.reshape(BC, HP * WP)
    up = np.zeros((BC, H * W), dtype=np.float16)
    up[np.arange(BC)[:, None], idx] = fv
    up = up.reshape(BC, H, W)
    # pad to 132 cols with NEG guards (cols 0:2 and 130:132) so each slab's
    # row-block is a single contiguous DMA per partition
    quads = []
    for rp in (0, 1):
        for cp in (0, 1):
            q = np.full((BC, HP, 132), NEG, dtype=np.float16)
            q[:, :, 2:130] = up[:, rp::2, cp::2]
            quads.append(q)
    qee, qeo, qoe, qoo = quads
    return [{"qee": qee[k * PPC:(k + 1) * PPC], "qeo": qeo[k * PPC:(k + 1) * PPC],
             "qoe": qoe[k * PPC:(k + 1) * PPC], "qoo": qoo[k * PPC:(k + 1) * PPC]}
            for k in range(NCORES)]


def _gather_out(res):
    """Re-interleave planar parity outputs into the full [B*C, H, W] canvas."""
    out = np.empty((NCORES * PPC, H, W), dtype=np.float16)
    for k in range(NCORES):
        eo = res.results[k]["oE"]
        oo = res.results[k]["oO"]
        dst = out[k * PPC:(k + 1) * PPC]
        dst[:, 0::2, 0::2] = eo[:, :, 0:WP]
        dst[:, 0::2, 1::2] = eo[:, :, WP:]
        dst[:, 1::2, 0::2] = oo[:, :, 0:WP]
        dst[:, 1::2, 1::2] = oo[:, :, WP:]
    return out


def kernel(**inputs):
    f = np.asarray(inputs["f"])
    p = np.asarray(inputs["provenance"])
    B, C = f.shape[:2]
    assert f.shape == (B, C, HP, WP) and B * C == NCORES * PPC

    nc = _get_nc()
    from concourse.bass_utils import run_bass_kernel_spmd
    in_maps = _prep_in_maps(f, p)
    res = run_bass_kernel_spmd(nc, in_maps, core_ids=list(range(NCORES)))
    out = _gather_out(res)
    return out.reshape(B, C, H, W).astype(np.float32)


# revision 21
# speedup vs baseline: 1.1711x; 1.1711x over previous
"""Trainium2 Bass kernel for CudaMorphUnpool2D (max-unpool scatter + 3x3 dilation).

Strategy:
  - 1024 (b,c) planes sharded 128/core across 8 NeuronCores (fully data parallel).
  - Host prep: the unpool scatter (pure data movement, last-writer-wins) is folded
    into input marshaling: the 256x256 canvas is built per plane with one numpy
    fancy-assignment and shipped as 4 parity-quadrant planes (even/odd row x
    even/odd col), fp16, padded to 132 columns with -inf guards so each slab's
    row-block is a single contiguous DMA descriptor per partition.
  - Device: separable 3x3 windowed max entirely with 2x-rate DVE tensor_tensor
    MAX ops (fp16, stride-1, 4B-aligned APs).  The 2-byte-misaligned column
    shifts are materialized by the Scalar (ACT) engine, which is otherwise idle.
    Outputs stay parity-planar (even rows / odd rows; cols planar within) and
    are re-interleaved on the host during the gather step.
  - Out-of-canvas window taps use -65504 (fp16 lowest) guards to exactly match
    the reference's -inf padding semantics at the borders.

Measured on trn2 (8 cores): 132.9us vs 338.3us baseline (2.55x), rel err 3.7e-4.
DVE busy ~111us (the structural floor for this decomposition: 3.09 MAX-op
elements per output element at 2 elem/cycle); DMA ~34MB/core ~94us aggregate.
"""
import os
import sys
import numpy as np
from contextlib import ExitStack

H, W = 256, 256
HP, WP = 128, 128
SI = 16                 # quadrant rows per slab (out rows per slab = 2*SI)
NSLAB = HP // SI
NCORES = 8
PPC = 128               # planes per core
NEG = -65504.0          # fp16 lowest: stands in for the reference's -inf pad

for _p in ("/opt/trn_rl_repo", "/root/.axon_site/_ro/trn_rl_repo"):
    if os.path.isdir(_p) and _p not in sys.path:
        sys.path.append(_p)


def _build_nc():
    import concourse.bass as bass  # noqa: F401
    import concourse.tile as tile
    from concourse import bacc, mybir

    dt = mybir.dt.float16
    AO = mybir.AluOpType

    nc = bacc.Bacc("TRN2", target_bir_lowering=False, debug=False)
    # quadrant canvases, host-padded to 132 cols (data in [2:130], NEG guards
    # elsewhere) so a whole slab row-block is one contiguous DMA per partition:
    # q<rowparity><colparity>[p, a, 2+b] = canvas[2a+rp, 2b+cp]
    qee = nc.dram_tensor("qee", [PPC, HP, 132], dt, kind="ExternalInput").ap()
    qeo = nc.dram_tensor("qeo", [PPC, HP, 132], dt, kind="ExternalInput").ap()
    qoe = nc.dram_tensor("qoe", [PPC, HP, 132], dt, kind="ExternalInput").ap()
    qoo = nc.dram_tensor("qoo", [PPC, HP, 132], dt, kind="ExternalInput").ap()
    # outputs: planar parity rows; cols 0:128 = even out cols, 128:256 = odd
    oE = nc.dram_tensor("oE", [PPC, HP, 2 * WP], dt, kind="ExternalOutput").ap()
    oO = nc.dram_tensor("oO", [PPC, HP, 2 * WP], dt, kind="ExternalOutput").ap()

    with tile.TileContext(nc) as tc, ExitStack() as ctx:
        pin = ctx.enter_context(tc.tile_pool(name="pin", bufs=2))
        psh = ctx.enter_context(tc.tile_pool(name="psh", bufs=2))
        pp = ctx.enter_context(tc.tile_pool(name="pp", bufs=2))
        pcm = ctx.enter_context(tc.tile_pool(name="pcm", bufs=2))
        ps = ctx.enter_context(tc.tile_pool(name="ps", bufs=2))
        pout = ctx.enter_context(tc.tile_pool(name="pout", bufs=2))

        for s in range(NSLAB):
            i0 = s * SI
            # --- input tiles: data in cols [2:130]; guard cols 0:2 & 130:132.
            # E-plane tile rows t=0..16  <->  quadrant row a = i0 + t
            # O-plane tile rows t=0..16  <->  quadrant row a = i0 - 1 + t
            QEE = pin.tile([128, SI + 1, 132], dt, tag="QEE")
            QEO = pin.tile([128, SI + 1, 132], dt, tag="QEO")
            QOE = pin.tile([128, SI + 1, 132], dt, tag="QOE")
            QOO = pin.tile([128, SI + 1, 132], dt, tag="QOO")
            # E rows: a in [i0, i0+SI]; last slab: a=HP row is out-of-canvas
            e_hi = min(HP, i0 + SI + 1)
            n_e = e_hi - i0
            if n_e < SI + 1:
                nc.gpsimd.memset(QEE[:, n_e:, :], NEG)
                nc.gpsimd.memset(QEO[:, n_e:, :], NEG)
            nc.sync.dma_start(QEE[:, 0:n_e, :], qee[:, i0:e_hi, :])
            nc.sync.dma_start(QEO[:, 0:n_e, :], qeo[:, i0:e_hi, :])
            # O rows: a in [i0-1, i0+SI-1]; first slab: a=-1 row is out-of-canvas
            o_lo = max(0, i0 - 1)
            t0 = o_lo - (i0 - 1)
            if t0 > 0:
                nc.gpsimd.memset(QOE[:, 0:t0, :], NEG)
                nc.gpsimd.memset(QOO[:, 0:t0, :], NEG)
            nc.sync.dma_start(QOE[:, t0:, :], qoe[:, o_lo:i0 + SI, :])
            nc.sync.dma_start(QOO[:, t0:, :], qoo[:, o_lo:i0 + SI, :])

            # --- ACT: 4B-realigned column-shifted copies (odd-elem offsets)
            shEO = psh.tile([128, SI + 1, 128], dt, tag="shEO")  # O[b-1], even rows
            shEE = psh.tile([128, SI + 1, 128], dt, tag="shEE")  # E[b+1], even rows
            shOO = psh.tile([128, SI + 1, 128], dt, tag="shOO")  # O[b-1], odd rows
            shOE = psh.tile([128, SI + 1, 128], dt, tag="shOE")  # E[b+1], odd rows
            nc.scalar.copy(shEO[:], QEO[:, :, 1:129])
            nc.scalar.copy(shEE[:], QEE[:, :, 3:131])
            nc.scalar.copy(shOO[:], QOO[:, :, 1:129])
            nc.scalar.copy(shOE[:], QOE[:, :, 3:131])

            # --- colmax (all DVE MAX at 2x): cm[p, t, 0:128]=even out cols,
            # cm[p, t, 128:256]=odd out cols
            P_e = pp.tile([128, SI + 1, 128], dt, tag="P_e")
            P_o = pp.tile([128, SI + 1, 128], dt, tag="P_o")
            nc.vector.tensor_tensor(P_e[:], QEE[:, :, 2:130], QEO[:, :, 2:130], AO.max)
            nc.vector.tensor_tensor(P_o[:], QOE[:, :, 2:130], QOO[:, :, 2:130], AO.max)
            cmE = pcm.tile([128, SI + 1, 256], dt, tag="cmE")
            cmO = pcm.tile([128, SI + 1, 256], dt, tag="cmO")
            nc.vector.tensor_tensor(cmE[:, :, 0:128], shEO[:], P_e[:], AO.max)
            nc.vector.tensor_tensor(cmE[:, :, 128:256], P_e[:], shEE[:], AO.max)
            nc.vector.tensor_tensor(cmO[:, :, 0:128], shOO[:], P_o[:], AO.max)
            nc.vector.tensor_tensor(cmO[:, :, 128:256], P_o[:], shOE[:], AO.max)

            # --- rowmax: out even row 2a = max(cmO[a-1], cmE[a], cmO[a])
            #             out odd  row 2a+1 = max(cmE[a], cmO[a], cmE[a+1])
            # tile idx: cmE[u] <-> a=i0+u ; cmO[u] <-> a=i0-1+u
            S = ps.tile([128, SI, 256], dt, tag="S")
            outE = pout.tile([128, SI, 256], dt, tag="outE")
            outO = pout.tile([128, SI, 256], dt, tag="outO")
            nc.vector.tensor_tensor(S[:], cmE[:, 0:SI, :], cmO[:, 1:SI + 1, :], AO.max)
            nc.vector.tensor_tensor(outE[:], cmO[:, 0:SI, :], S[:], AO.max)
            nc.vector.tensor_tensor(outO[:], S[:], cmE[:, 1:SI + 1, :], AO.max)

            nc.sync.dma_start(oE[:, i0:i0 + SI, :], outE[:])
            nc.sync.dma_start(oO[:, i0:i0 + SI, :], outO[:])

    nc.compile()
    return nc


_NC_CACHE = {}


def _get_nc():
    if "nc" not in _NC_CACHE:
        _NC_CACHE["nc"] = _build_nc()
    return _NC_CACHE["nc"]


def _prep_in_maps(f, p):
    """Host prep: unpool-scatter into the canvas (last-writer-wins, matching the
    reference's row-major duplicate-index semantics), split into parity
    quadrants, shard across cores."""
    BC = f.shape[0] * f.shape[1]
    fv = f.reshape(BC, HP * WP).astype(np.float16)
    idx = p.reshape(BC, HP * WP)
    up = np.zeros((BC, H * W), dtype=np.float16)
    up[np.arange(BC)[:, None], idx] = fv
    up = up.reshape(BC, H, W)
    # pad to 132 cols with NEG guards (cols 0:2 and 130:132) so each slab's
    # row-block is a single contiguous DMA per partition
    quads = []
    for rp in (0, 1):
        for cp in (0, 1):
            q = np.full((BC, HP, 132), NEG, dtype=np.float16)
            q[:, :, 2:130] = up[:, rp::2, cp::2]
            quads.append(q)
    qee, qeo, qoe, qoo = quads
    return [{"qee": qee[k * PPC:(k + 1) * PPC], "qeo": qeo[k * PPC:(k + 1) * PPC],
             "qoe": qoe[k * PPC:(k + 1) * PPC], "qoo": qoo[k * PPC:(k + 1) * PPC]}
            for k in range(NCORES)]


def _gather_out(res):
    """Re-interleave planar parity outputs into the full [B*C, H, W] canvas."""
    out = np.empty((NCORES * PPC, H, W), dtype=np.float16)
    for k in range(NCORES):
        eo = res.results[k]["oE"]
        oo = res.results[k]["oO"]
        dst = out[k * PPC:(k + 1) * PPC]
        dst[:, 0::2, 0::2] = eo[:, :, 0:WP]
        dst[:, 0::2, 1::2] = eo[:, :, WP:]
        dst[:, 1::2, 0::2] = oo[:, :, 0:WP]
        dst[:, 1::2, 1::2] = oo[:, :, WP:]
    return out


def kernel(**inputs):
    f = np.asarray(inputs["f"])
    p = np.asarray(inputs["provenance"])
    B, C = f.shape[:2]
    assert f.shape == (B, C, HP, WP) and B * C == NCORES * PPC

    nc = _get_nc()
    from concourse.bass_utils import run_bass_kernel_spmd
    in_maps = _prep_in_maps(f, p)
    res = run_bass_kernel_spmd(nc, in_maps, core_ids=list(range(NCORES)))
    out = _gather_out(res)
    return out.reshape(B, C, H, W).astype(np.float32)
